# revision 19
# baseline (speedup 1.0000x reference)
# Trainium2 Bass kernel for nn_DeConv2d (2x2 stride-2 deconvolution /
# pixel-shuffle) over inputs:
#   batches (32, 256, 64, 64) f32, weights (256, 256, 2, 2) f32,
#   biases  (256, 256, 2, 2) f32
# out[n,o,2h+k,2w+l] = sum_c x[n,c,h,w] * W[o,c,k,l] + sum_c b[o,c,k,l]
#
# Sharding: data-parallel over batch n across 8 NeuronCores (4 images each).
# Weights/biases replicated. No collectives.
#
# Per-core schedule: load each image's activations once (2x 1 MiB bf16
# tiles, c-chunk major). For each 8-input-row chunk, oC half and kernel
# offset, run a 2-step accumulating bf16 matmul (K=128x2, M=128, N=512,
# fp32 PSUM accumulate) into one PSUM bank, then evacuate PSUM->SBUF with
# stride-2 interleaved writes (fused fp32 bias add + fp32->bf16 cast,
# alternating ScalarE/VectorE) into a staging tile holding 32 complete
# output rows; DMA staging out as one contiguous 1 MiB store.
#
# Dtype choices: matmul operands bf16 (host-rounded; fp32 streams 4x
# slower through the PE and doubles activation HBM traffic); PSUM
# accumulate + bias fp32; OUTPUT STORED bf16 and upcast to fp32 on host
# after the gather (halves the dominant store traffic; adds ~1e-3 rel
# err on top of the ~2e-3 from bf16 matmul, vs a 2e-2 gate).
#
# Measured ~151 us/core, which is ~97% of the PE roofline on this part:
# the PE executes N=512 bf16 matmuls at ~283 ns each (= 512 rows at
# 2.4 GHz + ~70 ns/instruction overhead; equivalently 1.81 GHz
# effective), so 512 MMs/core = 145 us. HW-ablated components: this
# kernel's exact MM stream alone = 146.5 us, +evacuation = 148.0 us,
# full kernel ~151-153 us. The ~70 ns/MM gap was measured insensitive
# to: weight reuse (stationary vs per-MM reload), explicit
# InstLdweights + non-self-loading matmuls (ins.ldweights=False), flat
# 2D vs 3D moving APs, PSUM bank rotation/accumulation-pair pattern,
# and the presence of evac/store consumers -- it matches
# TRN2Spec.EXPECTED_SEQ_OVERHEAD_NS[PE]=71 (SW-decode sequencer
# overhead). HBM traffic is 40 MiB/core (8 in + 32 out) ~= 110-117 us
# at the ~358 GB/s per-NC limit, fully overlapped under the PE stream.
# Matmul N is ISA-capped at 512 fp32 PSUM elements
# (s3d3_mm_num_elements), so the per-MM overhead cannot be amortized
# further; fp8 DoubleRow would halve row-cycles but its ~5% (full) /
# ~3.6% (half-K split) quantization error fails the 2e-2 gate.

import numpy as np

N_CORES = 8
N_TOTAL, IC, IH, IW = 32, 256, 64, 64
OC, KH, KW = 256, 2, 2
NB = N_TOTAL // N_CORES  # images per core
HC_ROWS = 8              # input rows per matmul group -> N = 8*64 = 512
N_HC = IH // HC_ROWS     # matmul groups per image
HPAIR = 2                # matmul groups per staging tile / output DMA


def _emit_body(nc, x, y, w_tiles, b_tiles, xs, stage, psum, f32, bf16):
    orows = 2 * HPAIR * HC_ROWS      # output rows per staging tile
    for b in range(NB):
        # full-image activation tiles, one per c-chunk (1 MiB each, bf16).
        # SWDGE (gpsimd) so loads don't queue behind stores on the SP ring.
        xt = []
        for cc in range(2):
            t = xs.tile([128, IH, IW], bf16, tag=f"x_{cc}")
            nc.gpsimd.dma_start(out=t[:], in_=x[b, cc * 128:(cc + 1) * 128, :, :])
            xt.append(t)
        for hp in range(N_HC // HPAIR):
            for oc in range(2):
                st = stage.tile([128, orows, 2 * IW], bf16, tag="S")
                for hi in range(HPAIR):
                    h0 = (hp * HPAIR + hi) * HC_ROWS
                    for kl in range(4):
                        k, l = kl // 2, kl % 2
                        pt = psum.tile([128, HC_ROWS, IW], f32, tag="pt")
                        nc.tensor.matmul(
                            pt[:], w_tiles[kl, 0, oc][:],
                            xt[0][:, h0:h0 + HC_ROWS, :],
                            start=True, stop=False,
                        )
                        nc.tensor.matmul(
                            pt[:], w_tiles[kl, 1, oc][:],
                            xt[1][:, h0:h0 + HC_ROWS, :],
                            start=False, stop=True,
                        )
                        r0 = 2 * hi * HC_ROWS + k
                        dest = st[:, r0:r0 + 2 * HC_ROWS - 1:2, l:2 * IW:2]
                        bias_ap = b_tiles[oc][:, kl:kl + 1]
                        # fused bias add + fp32->bf16 cast + stride-2
                        # interleave, alternating ScalarE/VectorE
                        if l == 0:
                            nc.scalar.add(dest, pt[:], bias_ap)
                        else:
                            nc.vector.tensor_scalar_add(dest, pt[:], bias_ap)
                nc.sync.dma_start(
                    out=y[b, oc * 128:(oc + 1) * 128,
                          hp * orows:(hp + 1) * orows, :],
                    in_=st[:],
                )


def _build_bass(finalize=True, dyn_repeat=None):
    import contextlib

    import concourse.mybir as mybir
    import concourse.tile as tile
    from concourse import bacc

    f32 = mybir.dt.float32
    bf16 = mybir.dt.bfloat16
    nc = bacc.Bacc(None, target_bir_lowering=False)

    x = nc.dram_tensor("x", [NB, IC, IH, IW], bf16, kind="ExternalInput")
    wt = nc.dram_tensor("wt", [KH * KW, IC, OC], bf16, kind="ExternalInput")
    bs = nc.dram_tensor("bs", [OC, KH * KW], f32, kind="ExternalInput")
    # Output stored as bf16 (halves the dominant HBM store traffic; host
    # upcasts to fp32 after gather — adds ~1e-3 rel err, well within gate).
    y = nc.dram_tensor("y", [NB, OC, IH * KH, IW * KW], bf16, kind="ExternalOutput")

    with tile.TileContext(nc) as tc:
        with (
            tc.tile_pool(name="consts", bufs=1) as consts,
            tc.tile_pool(name="xs", bufs=2) as xs,
            tc.tile_pool(name="stage", bufs=4) as stage,
            tc.tile_pool(name="psum", bufs=8, space="PSUM") as psum,
        ):
            # Stationary weights: wT[kl][cc][oc] = [c(128 part), o(128 free)]
            w_tiles = {}
            for kl in range(4):
                for cc in range(2):
                    for oc in range(2):
                        t = consts.tile([128, 128], bf16, tag=f"w_{kl}_{cc}_{oc}")
                        nc.sync.dma_start(
                            out=t[:],
                            in_=wt[kl, cc * 128:(cc + 1) * 128, oc * 128:(oc + 1) * 128],
                        )
                        w_tiles[kl, cc, oc] = t
            # Per-oC-half bias columns: [o(128 part), kl(4)]
            b_tiles = {}
            for oc in range(2):
                t = consts.tile([128, 4], f32, tag=f"bs_{oc}")
                nc.sync.dma_start(out=t[:], in_=bs[oc * 128:(oc + 1) * 128, :])
                b_tiles[oc] = t

            loop_cm = (
                tc.For_i(0, dyn_repeat, 1)
                if dyn_repeat is not None
                else contextlib.nullcontext()
            )
            with loop_cm:
                _emit_body(nc, x, y, w_tiles, b_tiles, xs, stage, psum, f32, bf16)
    if finalize:
        nc.finalize()
    return nc


def _make_in_maps(batches, weights, biases):
    import ml_dtypes

    batches = np.asarray(batches, dtype=np.float32)
    weights = np.asarray(weights, dtype=np.float32)
    biases = np.asarray(biases, dtype=np.float32)

    # wT[kl, c, o] = W[o, c, k, l], rounded to bf16 (matmul operand dtype)
    wt = np.ascontiguousarray(
        weights.transpose(2, 3, 1, 0).reshape(KH * KW, IC, OC)
    ).astype(ml_dtypes.bfloat16)
    # bias summed over input channels (kept fp32): bs[o, kl]
    bs = np.ascontiguousarray(biases.sum(axis=1).reshape(OC, KH * KW))

    return [
        {
            "x": np.ascontiguousarray(batches[i * NB:(i + 1) * NB]).astype(
                ml_dtypes.bfloat16
            ),
            "wt": wt,
            "bs": bs,
        }
        for i in range(N_CORES)
    ]


def _prep_in_maps(seed=0):
    # Random same-shape inputs for the timing harness.
    rng = np.random.default_rng(seed)
    return _make_in_maps(
        rng.standard_normal((N_TOTAL, IC, IH, IW), dtype=np.float32),
        rng.standard_normal((OC, IC, KH, KW), dtype=np.float32),
        rng.standard_normal((OC, IC, KH, KW), dtype=np.float32),
    )


def kernel(batches, weights, biases):
    from concourse.bass_utils import run_bass_kernel_spmd

    nc = _build_bass()
    in_maps = _make_in_maps(batches, weights, biases)
    res = run_bass_kernel_spmd(nc, in_maps, core_ids=list(range(N_CORES)))
    return np.concatenate([r["y"] for r in res.results], axis=0).astype(np.float32)



# revision 20
# speedup vs baseline: 1.2405x; 1.2405x over previous
# Trainium2 Bass kernel for nn_DeConv2d (2x2 stride-2 deconvolution /
# pixel-shuffle) over inputs:
#   batches (32, 256, 64, 64) f32, weights (256, 256, 2, 2) f32,
#   biases  (256, 256, 2, 2) f32
# out[n,o,2h+k,2w+l] = sum_c x[n,c,h,w] * W[o,c,k,l] + sum_c b[o,c,k,l]
#
# Sharding: data-parallel over batch n across 8 NeuronCores (4 images each).
# Weights/biases replicated. No collectives.
#
# Per-core schedule: load each image's activations once (2x 1 MiB bf16
# tiles, c-chunk major). For each 8-input-row chunk, oC half and kernel
# offset, run a 2-step accumulating bf16 matmul (K=128x2, M=128, N=512,
# fp32 PSUM accumulate) into one PSUM bank, then evacuate PSUM->SBUF with
# stride-2 interleaved writes (fused fp32 bias add + fp32->bf16 cast,
# alternating ScalarE/VectorE) into a staging tile holding 32 complete
# output rows; DMA staging out as one contiguous 1 MiB store.
#
# Dtype choices: matmul operands bf16 (host-rounded; fp32 streams 4x
# slower through the PE and doubles activation HBM traffic); PSUM
# accumulate + bias fp32; OUTPUT STORED bf16 and upcast to fp32 on host
# after the gather (halves the dominant store traffic; adds ~1e-3 rel
# err on top of the ~2e-3 from bf16 matmul, vs a 2e-2 gate).
#
# Measured ~151 us/core, which is ~97% of the PE roofline on this part:
# the PE executes N=512 bf16 matmuls at ~283 ns each (= 512 rows at
# 2.4 GHz + ~70 ns/instruction overhead; equivalently 1.81 GHz
# effective), so 512 MMs/core = 145 us. HW-ablated components: this
# kernel's exact MM stream alone = 146.5 us, +evacuation = 148.0 us,
# full kernel ~151-153 us. The ~70 ns/MM gap was measured insensitive
# to: weight reuse (stationary vs per-MM reload), explicit
# InstLdweights + non-self-loading matmuls (ins.ldweights=False), flat
# 2D vs 3D moving APs, PSUM bank rotation/accumulation-pair pattern,
# and the presence of evac/store consumers -- it matches
# TRN2Spec.EXPECTED_SEQ_OVERHEAD_NS[PE]=71 (SW-decode sequencer
# overhead). HBM traffic is 40 MiB/core (8 in + 32 out) ~= 110-117 us
# at the ~358 GB/s per-NC limit, fully overlapped under the PE stream.
# Matmul N is ISA-capped at 512 fp32 PSUM elements
# (s3d3_mm_num_elements), so the per-MM overhead cannot be amortized
# further; fp8 DoubleRow would halve row-cycles but its ~5% (full) /
# ~3.6% (half-K split) quantization error fails the 2e-2 gate.

import numpy as np

N_CORES = 8
N_TOTAL, IC, IH, IW = 32, 256, 64, 64
OC, KH, KW = 256, 2, 2
NB = N_TOTAL // N_CORES  # images per core
HC_ROWS = 8              # input rows per matmul group -> N = 8*64 = 512
N_HC = IH // HC_ROWS     # matmul groups per image
HPAIR = 2                # matmul groups per staging tile / output DMA


def _emit_body(nc, x, y, w_tiles, b_tiles, xs, stage, psum, f32, bf16):
    orows = 2 * HPAIR * HC_ROWS      # output rows per staging tile
    for b in range(NB):
        # full-image activation tiles, one per c-chunk (1 MiB each, bf16).
        # SWDGE (gpsimd) so loads don't queue behind stores on the SP ring.
        xt = []
        for cc in range(2):
            t = xs.tile([128, IH, IW], bf16, tag=f"x_{cc}")
            nc.gpsimd.dma_start(out=t[:], in_=x[b, cc * 128:(cc + 1) * 128, :, :])
            xt.append(t)
        for hp in range(N_HC // HPAIR):
            for oc in range(2):
                st = stage.tile([128, orows, 2 * IW], bf16, tag="S")
                for hi in range(HPAIR):
                    h0 = (hp * HPAIR + hi) * HC_ROWS
                    for kl in range(4):
                        k, l = kl // 2, kl % 2
                        pt = psum.tile([128, HC_ROWS, IW], f32, tag="pt")
                        nc.tensor.matmul(
                            pt[:], w_tiles[kl, 0, oc][:],
                            xt[0][:, h0:h0 + HC_ROWS, :],
                            start=True, stop=False,
                        )
                        nc.tensor.matmul(
                            pt[:], w_tiles[kl, 1, oc][:],
                            xt[1][:, h0:h0 + HC_ROWS, :],
                            start=False, stop=True,
                        )
                        r0 = 2 * hi * HC_ROWS + k
                        dest = st[:, r0:r0 + 2 * HC_ROWS - 1:2, l:2 * IW:2]
                        bias_ap = b_tiles[oc][:, kl:kl + 1]
                        # fused bias add + fp32->bf16 cast + stride-2
                        # interleave, alternating ScalarE/VectorE
                        if l == 0:
                            nc.scalar.add(dest, pt[:], bias_ap)
                        else:
                            nc.vector.tensor_scalar_add(dest, pt[:], bias_ap)
                nc.sync.dma_start(
                    out=y[b, oc * 128:(oc + 1) * 128,
                          hp * orows:(hp + 1) * orows, :],
                    in_=st[:],
                )


def _build_bass(finalize=True, dyn_repeat=None):
    import contextlib

    import concourse.mybir as mybir
    import concourse.tile as tile
    from concourse import bacc

    f32 = mybir.dt.float32
    bf16 = mybir.dt.bfloat16
    nc = bacc.Bacc(None, target_bir_lowering=False)

    x = nc.dram_tensor("x", [NB, IC, IH, IW], bf16, kind="ExternalInput")
    wt = nc.dram_tensor("wt", [KH * KW, IC, OC], bf16, kind="ExternalInput")
    bs = nc.dram_tensor("bs", [OC, KH * KW], f32, kind="ExternalInput")
    # Output stored as bf16 (halves the dominant HBM store traffic; host
    # upcasts to fp32 after gather — adds ~1e-3 rel err, well within gate).
    y = nc.dram_tensor("y", [NB, OC, IH * KH, IW * KW], bf16, kind="ExternalOutput")

    with tile.TileContext(nc) as tc:
        with (
            tc.tile_pool(name="consts", bufs=1) as consts,
            tc.tile_pool(name="xs", bufs=2) as xs,
            tc.tile_pool(name="stage", bufs=3) as stage,
            tc.tile_pool(name="psum", bufs=8, space="PSUM") as psum,
        ):
            # Stationary weights: wT[kl][cc][oc] = [c(128 part), o(128 free)]
            w_tiles = {}
            for kl in range(4):
                for cc in range(2):
                    for oc in range(2):
                        t = consts.tile([128, 128], bf16, tag=f"w_{kl}_{cc}_{oc}")
                        nc.sync.dma_start(
                            out=t[:],
                            in_=wt[kl, cc * 128:(cc + 1) * 128, oc * 128:(oc + 1) * 128],
                        )
                        w_tiles[kl, cc, oc] = t
            # Per-oC-half bias columns: [o(128 part), kl(4)]
            b_tiles = {}
            for oc in range(2):
                t = consts.tile([128, 4], f32, tag=f"bs_{oc}")
                nc.sync.dma_start(out=t[:], in_=bs[oc * 128:(oc + 1) * 128, :])
                b_tiles[oc] = t

            loop_cm = (
                tc.For_i(0, dyn_repeat, 1)
                if dyn_repeat is not None
                else contextlib.nullcontext()
            )
            with loop_cm:
                _emit_body(nc, x, y, w_tiles, b_tiles, xs, stage, psum, f32, bf16)
    if finalize:
        nc.finalize()
    return nc


def _make_in_maps(batches, weights, biases):
    import ml_dtypes

    batches = np.asarray(batches, dtype=np.float32)
    weights = np.asarray(weights, dtype=np.float32)
    biases = np.asarray(biases, dtype=np.float32)

    # wT[kl, c, o] = W[o, c, k, l], rounded to bf16 (matmul operand dtype)
    wt = np.ascontiguousarray(
        weights.transpose(2, 3, 1, 0).reshape(KH * KW, IC, OC)
    ).astype(ml_dtypes.bfloat16)
    # bias summed over input channels (kept fp32): bs[o, kl]
    bs = np.ascontiguousarray(biases.sum(axis=1).reshape(OC, KH * KW))

    return [
        {
            "x": np.ascontiguousarray(batches[i * NB:(i + 1) * NB]).astype(
                ml_dtypes.bfloat16
            ),
            "wt": wt,
            "bs": bs,
        }
        for i in range(N_CORES)
    ]


def _prep_in_maps(seed=0):
    # Random same-shape inputs for the timing harness.
    rng = np.random.default_rng(seed)
    return _make_in_maps(
        rng.standard_normal((N_TOTAL, IC, IH, IW), dtype=np.float32),
        rng.standard_normal((OC, IC, KH, KW), dtype=np.float32),
        rng.standard_normal((OC, IC, KH, KW), dtype=np.float32),
    )


def kernel(batches, weights, biases):
    from concourse.bass_utils import run_bass_kernel_spmd

    nc = _build_bass()
    in_maps = _make_in_maps(batches, weights, biases)
    res = run_bass_kernel_spmd(nc, in_maps, core_ids=list(range(N_CORES)))
    return np.concatenate([r["y"] for r in res.results], axis=0).astype(np.float32)

